# revision 30
# baseline (speedup 1.0000x reference)
"""Trainium2 Bass kernel for nn_AttentionLayer (GQA attention layer, seq=2048,
hidden=4096, 32 Q heads / 8 KV heads, head_dim=128, causal).

Sharding: one GQA group (4 Q heads + 1 K + 1 V head) per NeuronCore (8 cores).
Each core computes its group's QKV projection, causal SDPA, and a partial
output projection over its 512 output-proj contraction dims; the host sums the
8 partials.

All matmuls run in bfloat16 with fp32 PSUM accumulation; inputs are converted
to bf16 on the host so DMA feeds the PE directly with no on-chip dtype
conversion. Attention uses the S^T layout: scores computed transposed
[s_k, s_q] so the PV matmul needs no P-tile transposes. V is projected
directly into [s, d] layout (tok tile as stationary), avoiding PE transposes.
Softmax denominators are computed incrementally per finalized 128-column
chunk (ones-vector matmul) so the per-head critical path has no wide serial
tail; exp runs on ACT, the running denominator sum on the Pool engine, and
the PV matmul trails the scores matmul by one k-tile so the PE never waits
on the exp chain. No max-subtraction (scores are O(5), exp is safe).
"""

import math

import numpy as np

SEQ = 2048
HIDDEN = 4096
HEAD_DIM = 128
N_CORES = 8
GROUP_PROJ = 768  # 4 Q heads + K + V, contiguous rows of weight_qkv per group
GROUP_E = 512  # 4 Q heads * head_dim: per-core slice of the proj contraction
SCALE = 1.0 / math.sqrt(HEAD_DIM)

_RUNNER = None


def _build_module(repeats=1):
    import concourse.bacc as bacc
    import concourse.mybir as mybir
    from concourse.tile import TileContext
    from concourse.masks import make_identity, make_upper_triangular

    dt = mybir.dt
    f32, bf16 = dt.float32, dt.bfloat16

    nc = bacc.Bacc(None, target_bir_lowering=False)
    tok_t = nc.declare_dram_parameter("tok_t", [HIDDEN, SEQ], bf16, isOutput=False)
    wq_t = nc.declare_dram_parameter("wq_t", [HIDDEN, GROUP_PROJ], bf16, isOutput=False)
    wp_t = nc.declare_dram_parameter("wp_t", [GROUP_E, HIDDEN], bf16, isOutput=False)
    out_part = nc.declare_dram_parameter("out_part", [SEQ, HIDDEN], f32, isOutput=True)

    with TileContext(nc) as tc:
        for _rep in range(repeats):
            _build_body(
                nc, tc, mybir, f32, bf16, make_identity, make_upper_triangular,
                tok_t, wq_t, wp_t, out_part,
            )

    nc.compile()
    return nc


def _build_body(
    nc, tc, mybir, f32, bf16, make_identity, make_upper_triangular,
    tok_t, wq_t, wp_t, out_part,
):
    Exp = mybir.ActivationFunctionType.Exp
    Copy = mybir.ActivationFunctionType.Copy
    mult = mybir.AluOpType.mult

    def copy_dve(out, in_):
        nc.vector.tensor_copy(out=out, in_=in_)

    def copy_act(out, in_):
        nc.scalar.activation(out=out, in_=in_, func=Copy)

    def copy_pool(out, in_):
        nc.gpsimd.tensor_copy(out=out, in_=in_)

    with (
        tc.tile_pool(name="persist", bufs=1) as persist,
        tc.tile_pool(name="late", bufs=1) as late_pool,
    ):
        # constants
        ident32 = persist.tile([128, 128], f32)
        make_identity(nc, ident32)
        identb = persist.tile([128, 128], bf16)
        nc.vector.tensor_copy(out=identb, in_=ident32)
        triu32 = persist.tile([128, 128], f32)
        make_upper_triangular(nc, triu32, val=1.0, diag=True)
        triub = persist.tile([128, 128], bf16)
        nc.vector.tensor_copy(out=triub, in_=triu32)
        ones32 = persist.tile([128, 1], f32)
        nc.gpsimd.memset(ones32, 1.0)
        ones_b = persist.tile([128, 1], bf16)
        nc.vector.tensor_copy(out=ones_b, in_=ones32)

        # persistent activations (bf16; qT/kT in [dim, seq], v in [seq, dim])
        qT = [
            [persist.tile([128, 512], bf16, name=f"qT{h}_{c}") for c in range(4)]
            for h in range(4)
        ]
        kT = [persist.tile([128, 128], bf16, name=f"kT{i}") for i in range(16)]
        v_sb = [persist.tile([128, 128], bf16, name=f"v{i}") for i in range(16)]

        # output-proj weights + attention outputs live across phases 2+3;
        # one wide tile per eo row-block, matmul'd from 512-col slices
        wp_wide = [
            late_pool.tile([128, HIDDEN], bf16, name=f"wpw{eo}") for eo in range(4)
        ]
        wp = [
            [wp_wide[eo][:, ck * 512 : (ck + 1) * 512] for ck in range(8)]
            for eo in range(4)
        ]
        aoT = [
            [late_pool.tile([128, 128], bf16, name=f"aoT{h}_{sti}") for sti in range(16)]
            for h in range(4)
        ]

        # ---- phase 1: QKV projection (qkv^T layout), v transposed via PE ----
        with (
            tc.tile_pool(name="wq", bufs=1) as wq_pool,
            tc.tile_pool(name="p1stage", bufs=6) as stage,
            tc.tile_pool(name="p1ps", bufs=1, space="PSUM") as p1ps,
            tc.tile_pool(name="tpps", bufs=2, space="PSUM") as tp_pool,
        ):
            wq_tiles = [
                wq_pool.tile([128, GROUP_PROJ], bf16, name=f"wq{kt}")
                for kt in range(32)
            ]
            pend_tp = None  # (sc, vTc) transposes deferred into the next sc

            def emit_transposes(pend):
                sc_, vTc_ = pend
                for j in range(4):
                    pst = tp_pool.tile([128, 128], bf16, tag="tp")
                    nc.tensor.transpose(
                        pst, vTc_[:, j * 128 : (j + 1) * 128], identb
                    )
                    copy_dve(v_sb[sc_ * 4 + j], pst)

            for sc in range(4):
                # ps[0..3]: q0..q3 transposed [d, s]; ps[4]: k; ps[5]: v (all
                # in [proj_dim, seq] layout)
                ps = [
                    p1ps.tile([128, 512], f32, tag=f"p1psum{pt}", name=f"p1ps{pt}_{sc}")
                    for pt in range(6)
                ]
                for kt in range(32):
                    st = stage.tile([128, 512], bf16, tag="tok_stage", bufs=6)
                    if sc == 0:
                        nc.sync.dma_start(
                            out=wq_tiles[kt], in_=wq_t[kt * 128 : (kt + 1) * 128, :]
                        )
                    nc.sync.dma_start(
                        out=st,
                        in_=tok_t[
                            kt * 128 : (kt + 1) * 128, sc * 512 : (sc + 1) * 512
                        ],
                    )
                    if kt == 2 and pend_tp is not None:
                        emit_transposes(pend_tp)
                        pend_tp = None
                    if sc > 0 and kt % 3 == 1:
                        # trickle-prefetch output-proj weights in [128,512]
                        # chunks so they never starve the token stream
                        ci = (sc - 1) * 11 + kt // 3
                        if ci < 32:
                            eo, ckk = divmod(ci, 8)
                            nc.sync.dma_start(
                                out=wp_wide[eo][:, ckk * 512 : (ckk + 1) * 512],
                                in_=wp_t[
                                    eo * 128 : (eo + 1) * 128,
                                    ckk * 512 : (ckk + 1) * 512,
                                ],
                            )
                    for pt in range(6):
                        nc.tensor.matmul(
                            ps[pt],
                            wq_tiles[kt][:, pt * 128 : (pt + 1) * 128],
                            st,
                            start=(kt == 0),
                            stop=(kt == 31),
                        )
                # evacuate PSUM -> bf16 SBUF, spread across DVE/ACT
                copy_dve(qT[0][sc], ps[0])
                copy_dve(qT[1][sc], ps[1])
                copy_act(qT[2][sc], ps[2])
                copy_act(qT[3][sc], ps[3])
                for j in range(4):
                    copy_dve(kT[sc * 4 + j], ps[4][:, j * 128 : (j + 1) * 128])
                vTc = stage.tile([128, 512], bf16, tag="vT_chunk", bufs=2)
                copy_act(vTc, ps[5])
                if sc < 3:
                    pend_tp = (sc, vTc)
                else:
                    emit_transposes((sc, vTc))

        # ---- phases 2+3 interleaved per q-chunk of 512 ----
        with (
            tc.tile_pool(name="attn", bufs=3) as attn_pool,
            tc.tile_pool(name="attps", bufs=2, space="PSUM") as attps,
            tc.tile_pool(name="aops", bufs=2, space="PSUM") as aops,
        ):
            pend_l = None  # (l_ps, ptr_r, linv_b, ao_ps, h, qg) awaiting emission

            def emit_l_tail(pend):
                # denominator matmul + normalization for a finished head;
                # called one head later so the PE never waits on the DVE chain
                ptr_r, linv_b, ao_ps, h_, qg_ = pend
                l_ps = attps.tile([128, 512], f32, tag="l", bufs=1)
                nc.tensor.matmul(l_ps[0:1, :], ones_b, ptr_r, start=True, stop=True)
                linv = attn_pool.tile([1, 512], f32, tag="linv", bufs=2)
                nc.vector.reciprocal(out=linv, in_=l_ps[0:1, :])
                nc.gpsimd.partition_broadcast(out_ap=linv_b, in_ap=linv)
                for j in range(4):
                    nc.vector.tensor_tensor(
                        aoT[h_][qg_ * 4 + j],
                        ao_ps[:, j * 128 : (j + 1) * 128],
                        linv_b[:, j * 128 : (j + 1) * 128],
                        mult,
                    )

            ph3_pend = None

            def emit_phase3(qg_):
                for j in range(4):
                    st_i = qg_ * 4 + j
                    for half in range(2):
                        osb = attn_pool.tile([128, 2048], f32, tag="osb", bufs=3)
                        for q in range(4):
                            ck = half * 4 + q
                            ops = aops.tile([128, 512], f32, tag="o", bufs=2)
                            for h_ in range(4):
                                nc.tensor.matmul(
                                    ops,
                                    aoT[h_][st_i],
                                    wp[h_][ck],
                                    start=(h_ == 0),
                                    stop=(h_ == 3),
                                )
                            (copy_dve if ck % 2 == 0 else copy_act)(
                                osb[:, q * 512 : (q + 1) * 512], ops
                            )
                        if st_i == 15:
                            # split the final writes so the end-of-kernel DMA
                            # drain is shorter
                            for piece in range(2):
                                nc.sync.dma_start(
                                    out=out_part[
                                        st_i * 128 : (st_i + 1) * 128,
                                        half * 2048
                                        + piece * 1024 : half * 2048
                                        + (piece + 1) * 1024,
                                    ],
                                    in_=osb[:, piece * 1024 : (piece + 1) * 1024],
                                )
                        else:
                            nc.sync.dma_start(
                                out=out_part[
                                    st_i * 128 : (st_i + 1) * 128,
                                    half * 2048 : (half + 1) * 2048,
                                ],
                                in_=osb,
                            )

            for qg in range(4):
                for h in range(4):
                    nkt = 4 * (qg + 1)
                    ao_ps = aops.tile([128, 512], f32, tag="ao")
                    ptot = attn_pool.tile([128, 512], bf16, tag="ptot", bufs=2)
                    linv_b = attn_pool.tile([128, 512], f32, tag="linvb")
                    pTs = []
                    for kt in range(nkt):
                        t = kt - 4 * qg
                        c0 = max(t, 0) * 128  # first valid s_q column
                        s_ps = attps.tile([128, 512], f32, tag="s", bufs=3)
                        nc.tensor.matmul(
                            s_ps[:, c0:],
                            kT[kt],
                            qT[h][qg][:, c0:] if c0 else qT[h][qg],
                            start=True,
                            stop=True,
                        )
                        if kt == min(2, nkt - 1) and pend_l is not None:
                            emit_l_tail(pend_l)
                            pend_l = None
                        pT = attn_pool.tile([128, 512], bf16, tag="pT", bufs=6)
                        nc.scalar.activation(
                            out=pT[:, c0:], in_=s_ps[:, c0:], func=Exp, scale=SCALE
                        )
                        if t >= 0:
                            # triangle mask on the diagonal 128-col block
                            nc.vector.tensor_tensor(
                                pT[:, c0 : c0 + 128],
                                pT[:, c0 : c0 + 128],
                                triub,
                                mult,
                            )
                        # running denominator sum on DVE
                        if kt == 0:
                            nc.vector.tensor_copy(out=ptot, in_=pT)
                        else:
                            nc.vector.tensor_add(
                                out=ptot[:, c0:], in0=ptot[:, c0:], in1=pT[:, c0:]
                            )
                        # PV trails scores by one k-tile
                        if kt >= 1:
                            km = kt - 1
                            c0m = max(km - 4 * qg, 0) * 128
                            nc.tensor.matmul(
                                ao_ps[:, c0m:],
                                v_sb[km],
                                pTs[km][:, c0m:],
                                start=(km == 0),
                                stop=False,
                            )
                        pTs.append(pT)
                    km = nkt - 1
                    nc.tensor.matmul(
                        ao_ps[:, 384:],
                        v_sb[km],
                        pTs[km][:, 384:],
                        start=False,
                        stop=True,
                    )
                    pend_l = (ptot, linv_b, ao_ps, h, qg)
                    if h == 0 and ph3_pend is not None:
                        # previous q-chunk's output projection, deferred until
                        # now so its aoT evacuation chain had a full head of
                        # slack and the PE stream never stalls on it
                        emit_phase3(ph3_pend)
                        ph3_pend = None

                ph3_pend = qg

            emit_l_tail(pend_l)
            pend_l = None
            emit_phase3(ph3_pend)
            ph3_pend = None



class _Runner:
    """Persistent jitted multi-core executor (clone of run_bass_via_pjrt)."""

    def __init__(self, nc, n_cores):
        import jax
        from jax.sharding import Mesh, PartitionSpec
        from jax.experimental.shard_map import shard_map
        import concourse.mybir as mybir
        from concourse import bass2jax

        bass2jax.install_neuronx_cc_hook()
        self.jax = jax
        self.n_cores = n_cores
        partition_name = (
            nc.partition_id_tensor.name if nc.partition_id_tensor else None
        )
        in_names, out_names, out_avals, zero_outs = [], [], [], []
        for alloc in nc.m.functions[0].allocations:
            if not isinstance(alloc, mybir.MemoryLocationSet):
                continue
            name = alloc.memorylocations[0].name
            if alloc.kind == "ExternalInput":
                if name != partition_name:
                    in_names.append(name)
            elif alloc.kind == "ExternalOutput":
                out_names.append(name)
                shape = tuple(alloc.tensor_shape)
                dtype = mybir.dt.np(alloc.dtype)
                out_avals.append(jax.core.ShapedArray(shape, dtype))
                zero_outs.append(np.zeros(shape, dtype))
        self.in_names = list(in_names)
        self.out_names = out_names
        self.out_avals = out_avals
        self.zero_outs = zero_outs
        n_params = len(in_names)
        n_outs = len(out_avals)
        all_in_names = in_names + out_names
        if partition_name is not None:
            all_in_names.append(partition_name)

        def _body(*args):
            operands = list(args)
            if partition_name is not None:
                operands.append(bass2jax.partition_id_tensor())
            outs = bass2jax._bass_exec_p.bind(
                *operands,
                out_avals=tuple(out_avals),
                in_names=tuple(all_in_names),
                out_names=tuple(out_names),
                lowering_input_output_aliases=(),
                sim_require_finite=True,
                sim_require_nnan=True,
                nc=nc,
            )
            return tuple(outs)

        self._body = _body
        self.n_params = n_params
        self.n_outs = n_outs
        devices = jax.devices()[:n_cores]
        self.mesh = Mesh(np.asarray(devices), ("core",))
        in_specs = (PartitionSpec("core"),) * (n_params + n_outs)
        out_specs = (PartitionSpec("core"),) * n_outs
        self.sharded = jax.jit(
            shard_map(
                _body,
                mesh=self.mesh,
                in_specs=in_specs,
                out_specs=out_specs,
                check_rep=False,
            ),
            donate_argnums=tuple(range(n_params, n_params + n_outs)),
            keep_unused=True,
        )

    def run(self, in_maps):
        concat_in = [
            np.concatenate(
                [np.asarray(in_maps[c][nm]) for c in range(self.n_cores)], axis=0
            )
            for nm in self.in_names
        ]
        zeros = [
            np.zeros((self.n_cores * z.shape[0], *z.shape[1:]), z.dtype)
            for z in self.zero_outs
        ]
        out_arrs = self.sharded(*concat_in, *zeros)
        return [
            {
                nm: np.asarray(out_arrs[i]).reshape(
                    self.n_cores, *self.out_avals[i].shape
                )[c]
                for i, nm in enumerate(self.out_names)
            }
            for c in range(self.n_cores)
        ]


def _get_runner():
    global _RUNNER
    if _RUNNER is None:
        nc = _build_module()
        _RUNNER = _Runner(nc, N_CORES)
    return _RUNNER


def make_in_maps(tokens, weight_qkv, weight_proj):
    """Host-side sharding: bf16-convert + transpose, one map per core."""
    import ml_dtypes

    bf16 = ml_dtypes.bfloat16
    tok_t = np.ascontiguousarray(
        np.asarray(tokens, dtype=np.float32).reshape(SEQ, HIDDEN).T
    ).astype(bf16)
    weight_qkv = np.asarray(weight_qkv, dtype=np.float32)
    weight_proj = np.asarray(weight_proj, dtype=np.float32)
    in_maps = []
    for g in range(N_CORES):
        wq_slice = weight_qkv[g * GROUP_PROJ : (g + 1) * GROUP_PROJ, :]
        wp_slice = weight_proj[:, g * GROUP_E : (g + 1) * GROUP_E]
        in_maps.append(
            {
                "tok_t": tok_t,
                "wq_t": np.ascontiguousarray(wq_slice.T).astype(bf16),
                "wp_t": np.ascontiguousarray(wp_slice.T).astype(bf16),
            }
        )
    return in_maps


def kernel(tokens, weight_qkv, weight_proj):
    runner = _get_runner()
    outs = runner.run(make_in_maps(tokens, weight_qkv, weight_proj))
    acc = outs[0]["out_part"].astype(np.float64)
    for c in range(1, N_CORES):
        acc += outs[c]["out_part"]
    return acc.astype(np.float32).reshape(SEQ, 1, HIDDEN)


# revision 32
# speedup vs baseline: 1.0476x; 1.0476x over previous
"""Trainium2 Bass kernel for nn_AttentionLayer (GQA attention layer, seq=2048,
hidden=4096, 32 Q heads / 8 KV heads, head_dim=128, causal).

Sharding: one GQA group (4 Q heads + 1 K + 1 V head) per NeuronCore (8 cores).
Each core computes its group's QKV projection, causal SDPA, and a partial
output projection over its 512 output-proj contraction dims; the host sums the
8 partials.

All matmuls run in bfloat16 with fp32 PSUM accumulation; inputs are converted
to bf16 on the host so DMA feeds the PE directly with no on-chip dtype
conversion. Attention uses the S^T layout: scores computed transposed
[s_k, s_q] so the PV matmul needs no P-tile transposes. V is projected
directly into [s, d] layout (tok tile as stationary), avoiding PE transposes.
Softmax denominators are computed incrementally per finalized 128-column
chunk (ones-vector matmul) so the per-head critical path has no wide serial
tail; exp runs on ACT, the running denominator sum on the Pool engine, and
the PV matmul trails the scores matmul by one k-tile so the PE never waits
on the exp chain. No max-subtraction (scores are O(5), exp is safe).
"""

import math

import numpy as np

SEQ = 2048
HIDDEN = 4096
HEAD_DIM = 128
N_CORES = 8
GROUP_PROJ = 768  # 4 Q heads + K + V, contiguous rows of weight_qkv per group
GROUP_E = 512  # 4 Q heads * head_dim: per-core slice of the proj contraction
SCALE = 1.0 / math.sqrt(HEAD_DIM)

_RUNNER = None


def _build_module(repeats=1):
    import concourse.bacc as bacc
    import concourse.mybir as mybir
    from concourse.tile import TileContext
    from concourse.masks import make_identity, make_upper_triangular

    dt = mybir.dt
    f32, bf16 = dt.float32, dt.bfloat16

    nc = bacc.Bacc(None, target_bir_lowering=False)
    tok_t = nc.declare_dram_parameter("tok_t", [HIDDEN, SEQ], bf16, isOutput=False)
    wq_t = nc.declare_dram_parameter("wq_t", [HIDDEN, GROUP_PROJ], bf16, isOutput=False)
    wp_t = nc.declare_dram_parameter("wp_t", [GROUP_E, HIDDEN], bf16, isOutput=False)
    out_part = nc.declare_dram_parameter("out_part", [SEQ, HIDDEN], f32, isOutput=True)

    with TileContext(nc) as tc:
        for _rep in range(repeats):
            _build_body(
                nc, tc, mybir, f32, bf16, make_identity, make_upper_triangular,
                tok_t, wq_t, wp_t, out_part,
            )

    nc.compile()
    return nc


def _build_body(
    nc, tc, mybir, f32, bf16, make_identity, make_upper_triangular,
    tok_t, wq_t, wp_t, out_part,
):
    Exp = mybir.ActivationFunctionType.Exp
    Copy = mybir.ActivationFunctionType.Copy
    mult = mybir.AluOpType.mult

    def copy_dve(out, in_):
        nc.vector.tensor_copy(out=out, in_=in_)

    def copy_act(out, in_):
        nc.scalar.activation(out=out, in_=in_, func=Copy)

    def copy_pool(out, in_):
        nc.gpsimd.tensor_copy(out=out, in_=in_)

    with (
        tc.tile_pool(name="persist", bufs=1) as persist,
        tc.tile_pool(name="late", bufs=1) as late_pool,
    ):
        # constants
        ident32 = persist.tile([128, 128], f32)
        make_identity(nc, ident32)
        identb = persist.tile([128, 128], bf16)
        nc.vector.tensor_copy(out=identb, in_=ident32)
        triu32 = persist.tile([128, 128], f32)
        make_upper_triangular(nc, triu32, val=1.0, diag=True)
        triub = persist.tile([128, 128], bf16)
        nc.vector.tensor_copy(out=triub, in_=triu32)
        ones32 = persist.tile([128, 1], f32)
        nc.gpsimd.memset(ones32, 1.0)
        ones_b = persist.tile([128, 1], bf16)
        nc.vector.tensor_copy(out=ones_b, in_=ones32)

        # persistent activations (bf16; qT/kT in [dim, seq], v in [seq, dim])
        qT = [
            [persist.tile([128, 512], bf16, name=f"qT{h}_{c}") for c in range(4)]
            for h in range(4)
        ]
        kT = [persist.tile([128, 128], bf16, name=f"kT{i}") for i in range(16)]
        v_sb = [persist.tile([128, 128], bf16, name=f"v{i}") for i in range(16)]

        # output-proj weights + attention outputs live across phases 2+3;
        # one wide tile per eo row-block, matmul'd from 512-col slices
        wp_wide = [
            late_pool.tile([128, HIDDEN], bf16, name=f"wpw{eo}") for eo in range(4)
        ]
        wp = [
            [wp_wide[eo][:, ck * 512 : (ck + 1) * 512] for ck in range(8)]
            for eo in range(4)
        ]
        aoT = [
            [late_pool.tile([128, 128], bf16, name=f"aoT{h}_{sti}") for sti in range(16)]
            for h in range(4)
        ]

        # ---- phase 1: QKV projection (qkv^T layout), v transposed via PE ----
        with (
            tc.tile_pool(name="wq", bufs=1) as wq_pool,
            tc.tile_pool(name="p1stage", bufs=6) as stage,
            tc.tile_pool(name="p1ps", bufs=1, space="PSUM") as p1ps,
            tc.tile_pool(name="tpps", bufs=2, space="PSUM") as tp_pool,
        ):
            wq_tiles = [
                wq_pool.tile([128, GROUP_PROJ], bf16, name=f"wq{kt}")
                for kt in range(32)
            ]
            pend_tp = None  # (sc, vTc) transposes deferred into the next sc

            def emit_transposes(pend):
                sc_, vTc_ = pend
                for j in range(4):
                    pst = tp_pool.tile([128, 128], bf16, tag="tp")
                    nc.tensor.transpose(
                        pst, vTc_[:, j * 128 : (j + 1) * 128], identb
                    )
                    copy_dve(v_sb[sc_ * 4 + j], pst)

            for sc in range(4):
                # ps[0..3]: q0..q3 transposed [d, s]; ps[4]: k; ps[5]: v (all
                # in [proj_dim, seq] layout)
                ps = [
                    p1ps.tile([128, 512], f32, tag=f"p1psum{pt}", name=f"p1ps{pt}_{sc}")
                    for pt in range(6)
                ]
                for kt in range(32):
                    st = stage.tile([128, 512], bf16, tag="tok_stage", bufs=6)
                    if sc == 0:
                        nc.sync.dma_start(
                            out=wq_tiles[kt], in_=wq_t[kt * 128 : (kt + 1) * 128, :]
                        )
                    nc.sync.dma_start(
                        out=st,
                        in_=tok_t[
                            kt * 128 : (kt + 1) * 128, sc * 512 : (sc + 1) * 512
                        ],
                    )
                    if kt == 2 and pend_tp is not None:
                        emit_transposes(pend_tp)
                        pend_tp = None
                    if sc > 0 and kt % 3 == 1:
                        # trickle-prefetch output-proj weights in [128,512]
                        # chunks so they never starve the token stream
                        ci = (sc - 1) * 11 + kt // 3
                        if ci < 32:
                            eo, ckk = divmod(ci, 8)
                            nc.sync.dma_start(
                                out=wp_wide[eo][:, ckk * 512 : (ckk + 1) * 512],
                                in_=wp_t[
                                    eo * 128 : (eo + 1) * 128,
                                    ckk * 512 : (ckk + 1) * 512,
                                ],
                            )
                    for pt in range(6):
                        nc.tensor.matmul(
                            ps[pt],
                            wq_tiles[kt][:, pt * 128 : (pt + 1) * 128],
                            st,
                            start=(kt == 0),
                            stop=(kt == 31),
                        )
                # evacuate PSUM -> bf16 SBUF, spread across DVE/ACT
                copy_dve(qT[0][sc], ps[0])
                copy_dve(qT[1][sc], ps[1])
                copy_act(qT[2][sc], ps[2])
                copy_act(qT[3][sc], ps[3])
                for j in range(4):
                    copy_dve(kT[sc * 4 + j], ps[4][:, j * 128 : (j + 1) * 128])
                vTc = stage.tile([128, 512], bf16, tag="vT_chunk", bufs=2)
                copy_act(vTc, ps[5])
                if sc < 3:
                    pend_tp = (sc, vTc)
                else:
                    emit_transposes((sc, vTc))

        # ---- phases 2+3 interleaved per q-chunk of 512 ----
        with (
            tc.tile_pool(name="attn", bufs=3) as attn_pool,
            tc.tile_pool(name="attps", bufs=2, space="PSUM") as attps,
            tc.tile_pool(name="aops", bufs=2, space="PSUM") as aops,
        ):
            pend_l = None  # (l_ps, ptr_r, linv_b, ao_ps, h, qg) awaiting emission

            def emit_l_tail(pend):
                # denominator matmul + normalization for a finished head;
                # called one head later so the PE never waits on the DVE chain
                ptr_r, linv_b, ao_ps, h_, qg_ = pend
                l_ps = attps.tile([128, 512], f32, tag="l", bufs=1)
                nc.tensor.matmul(l_ps[0:1, :], ones_b, ptr_r, start=True, stop=True)
                linv = attn_pool.tile([1, 512], f32, tag="linv", bufs=2)
                nc.vector.reciprocal(out=linv, in_=l_ps[0:1, :])
                nc.gpsimd.partition_broadcast(out_ap=linv_b, in_ap=linv)
                for j in range(4):
                    nc.vector.tensor_tensor(
                        aoT[h_][qg_ * 4 + j],
                        ao_ps[:, j * 128 : (j + 1) * 128],
                        linv_b[:, j * 128 : (j + 1) * 128],
                        mult,
                    )

            ph3_pend = None

            def emit_phase3(qg_):
                for j in range(4):
                    st_i = qg_ * 4 + j
                    for half in range(2):
                        osb = attn_pool.tile([128, 2048], f32, tag="osb", bufs=3)
                        for q in range(4):
                            ck = half * 4 + q
                            ops = aops.tile([128, 512], f32, tag="o", bufs=2)
                            for h_ in range(4):
                                nc.tensor.matmul(
                                    ops,
                                    aoT[h_][st_i],
                                    wp[h_][ck],
                                    start=(h_ == 0),
                                    stop=(h_ == 3),
                                )
                            (copy_dve if ck % 2 == 0 else copy_act)(
                                osb[:, q * 512 : (q + 1) * 512], ops
                            )
                        if st_i == 15:
                            # split the final writes so the end-of-kernel DMA
                            # drain is shorter
                            for piece in range(2):
                                nc.sync.dma_start(
                                    out=out_part[
                                        st_i * 128 : (st_i + 1) * 128,
                                        half * 2048
                                        + piece * 1024 : half * 2048
                                        + (piece + 1) * 1024,
                                    ],
                                    in_=osb[:, piece * 1024 : (piece + 1) * 1024],
                                )
                        else:
                            nc.sync.dma_start(
                                out=out_part[
                                    st_i * 128 : (st_i + 1) * 128,
                                    half * 2048 : (half + 1) * 2048,
                                ],
                                in_=osb,
                            )

            for qg in range(4):
                for h in range(4):
                    nkt = 4 * (qg + 1)
                    ao_ps = aops.tile([128, 512], f32, tag="ao")
                    ptot = attn_pool.tile([128, 512], bf16, tag="ptot", bufs=2)
                    linv_b = attn_pool.tile([128, 512], f32, tag="linvb")
                    pTs = []
                    for kt in range(nkt):
                        t = kt - 4 * qg
                        c0 = max(t, 0) * 128  # first valid s_q column
                        s_ps = attps.tile([128, 512], f32, tag="s", bufs=3)
                        nc.tensor.matmul(
                            s_ps[:, c0:],
                            kT[kt],
                            qT[h][qg][:, c0:] if c0 else qT[h][qg],
                            start=True,
                            stop=True,
                        )
                        if kt == min(2, nkt - 1) and pend_l is not None:
                            emit_l_tail(pend_l)
                            pend_l = None
                        pT = attn_pool.tile([128, 512], bf16, tag="pT", bufs=6)
                        nc.scalar.activation(
                            out=pT[:, c0:], in_=s_ps[:, c0:], func=Exp, scale=SCALE
                        )
                        if t >= 0:
                            # triangle mask on the diagonal 128-col block
                            nc.vector.tensor_tensor(
                                pT[:, c0 : c0 + 128],
                                pT[:, c0 : c0 + 128],
                                triub,
                                mult,
                            )
                        # running denominator sum on DVE
                        if kt == 0:
                            nc.vector.tensor_copy(out=ptot, in_=pT)
                        else:
                            nc.vector.tensor_add(
                                out=ptot[:, c0:], in0=ptot[:, c0:], in1=pT[:, c0:]
                            )
                        # PV trails scores by one k-tile
                        if kt >= 1:
                            km = kt - 1
                            c0m = max(km - 4 * qg, 0) * 128
                            nc.tensor.matmul(
                                ao_ps[:, c0m:],
                                v_sb[km],
                                pTs[km][:, c0m:],
                                start=(km == 0),
                                stop=False,
                            )
                        pTs.append(pT)
                    km = nkt - 1
                    nc.tensor.matmul(
                        ao_ps[:, 384:],
                        v_sb[km],
                        pTs[km][:, 384:],
                        start=False,
                        stop=True,
                    )
                    pend_l = (ptot, linv_b, ao_ps, h, qg)
                    if h == 0 and ph3_pend is not None:
                        # previous q-chunk's output projection, deferred until
                        # now so its aoT evacuation chain had a full head of
                        # slack and the PE stream never stalls on it
                        emit_phase3(ph3_pend)
                        ph3_pend = None

                ph3_pend = qg

            emit_l_tail(pend_l)
            pend_l = None
            emit_phase3(ph3_pend)
            ph3_pend = None



class _Runner:
    """Persistent jitted multi-core executor (clone of run_bass_via_pjrt)."""

    def __init__(self, nc, n_cores):
        import jax
        from jax.sharding import Mesh, PartitionSpec
        from jax.experimental.shard_map import shard_map
        import concourse.mybir as mybir
        from concourse import bass2jax

        bass2jax.install_neuronx_cc_hook()
        self.jax = jax
        self.n_cores = n_cores
        partition_name = (
            nc.partition_id_tensor.name if nc.partition_id_tensor else None
        )
        in_names, out_names, out_avals, zero_outs = [], [], [], []
        for alloc in nc.m.functions[0].allocations:
            if not isinstance(alloc, mybir.MemoryLocationSet):
                continue
            name = alloc.memorylocations[0].name
            if alloc.kind == "ExternalInput":
                if name != partition_name:
                    in_names.append(name)
            elif alloc.kind == "ExternalOutput":
                out_names.append(name)
                shape = tuple(alloc.tensor_shape)
                dtype = mybir.dt.np(alloc.dtype)
                out_avals.append(jax.core.ShapedArray(shape, dtype))
                zero_outs.append(np.zeros(shape, dtype))
        self.in_names = list(in_names)
        self.out_names = out_names
        self.out_avals = out_avals
        self.zero_outs = zero_outs
        n_params = len(in_names)
        n_outs = len(out_avals)
        all_in_names = in_names + out_names
        if partition_name is not None:
            all_in_names.append(partition_name)

        def _body(*args):
            operands = list(args)
            if partition_name is not None:
                operands.append(bass2jax.partition_id_tensor())
            outs = bass2jax._bass_exec_p.bind(
                *operands,
                out_avals=tuple(out_avals),
                in_names=tuple(all_in_names),
                out_names=tuple(out_names),
                lowering_input_output_aliases=(),
                sim_require_finite=True,
                sim_require_nnan=True,
                nc=nc,
            )
            return tuple(outs)

        self._body = _body
        self.n_params = n_params
        self.n_outs = n_outs
        devices = jax.devices()[:n_cores]
        self.mesh = Mesh(np.asarray(devices), ("core",))
        in_specs = (PartitionSpec("core"),) * (n_params + n_outs)
        out_specs = (PartitionSpec("core"),) * n_outs
        self.sharded = jax.jit(
            shard_map(
                _body,
                mesh=self.mesh,
                in_specs=in_specs,
                out_specs=out_specs,
                check_rep=False,
            ),
            donate_argnums=tuple(range(n_params, n_params + n_outs)),
            keep_unused=True,
        )

    def run(self, in_maps):
        concat_in = [
            np.concatenate(
                [np.asarray(in_maps[c][nm]) for c in range(self.n_cores)], axis=0
            )
            for nm in self.in_names
        ]
        zeros = [
            np.zeros((self.n_cores * z.shape[0], *z.shape[1:]), z.dtype)
            for z in self.zero_outs
        ]
        out_arrs = self.sharded(*concat_in, *zeros)
        return [
            {
                nm: np.asarray(out_arrs[i]).reshape(
                    self.n_cores, *self.out_avals[i].shape
                )[c]
                for i, nm in enumerate(self.out_names)
            }
            for c in range(self.n_cores)
        ]


def _get_runner():
    global _RUNNER
    if _RUNNER is None:
        nc = _build_module()
        _RUNNER = _Runner(nc, N_CORES)
    return _RUNNER


def make_in_maps(tokens, weight_qkv, weight_proj):
    """Host-side sharding: bf16-convert + transpose, one map per core."""
    import ml_dtypes

    bf16 = ml_dtypes.bfloat16
    tok_t = np.ascontiguousarray(
        np.asarray(tokens, dtype=np.float32).reshape(SEQ, HIDDEN).T
    ).astype(bf16)
    weight_qkv = np.asarray(weight_qkv, dtype=np.float32)
    weight_proj = np.asarray(weight_proj, dtype=np.float32)
    in_maps = []
    for g in range(N_CORES):
        wq_slice = weight_qkv[g * GROUP_PROJ : (g + 1) * GROUP_PROJ, :]
        wp_slice = weight_proj[:, g * GROUP_E : (g + 1) * GROUP_E]
        in_maps.append(
            {
                "tok_t": tok_t,
                "wq_t": np.ascontiguousarray(wq_slice.T).astype(bf16),
                "wp_t": np.ascontiguousarray(wp_slice.T).astype(bf16),
            }
        )
    return in_maps


def kernel(tokens, weight_qkv, weight_proj):
    runner = _get_runner()
    outs = runner.run(make_in_maps(tokens, weight_qkv, weight_proj))
    acc = outs[0]["out_part"].astype(np.float64)
    for c in range(1, N_CORES):
        acc += outs[c]["out_part"]
    return acc.astype(np.float32).reshape(SEQ, 1, HIDDEN)
